# revision 15
# baseline (speedup 1.0000x reference)
"""Trainium2 Bass kernel for nn_MultiHeadDotProductAttention_14980845928960.

Block-local multi-head attention with partial RoPE:
  q/k/v projections -> RoPE on first 32 of 64 head dims -> softmax(QK^T/8)V
  -> output projection.  Shapes: inputs [4,16,256,1024], 16 heads x 64 dim,
  blocks of 256 tokens attend locally.

Strategy: data-parallel over the 64 (batch, block) pairs -> 8 blocks/core.
Projections are batched over PAIRS of blocks (512 tokens -> N=512 moving
operands); attention runs per 256-token block but with both blocks of a
pair packed into [128, 512] tiles.  The emission order SOFTWARE-PIPELINES
pairs: pair p's ScalarE-heavy attention chunks are interleaved between
pair p+1's PE-heavy projection phases, so the tensor engine never idles
long enough for the HAM clock gate to re-throttle it.
Everything keeps the contraction dim on SBUF partitions:
  - inputs DMA'd with fp32->bf16 cast (gpsimd SWDGE); x^T via REGULAR bf16
    matmuls against a bf16 identity (lhsT = x chunk) -- all-bf16 PE keeps
    fast-weight-load enabled; projections as lhsT=W chunk, rhs=x^T chunk.
  - Q/K channel-PERMUTED (host side) so rope dims occupy out-chunks 0-3
    and pass dims chunks 4-7; RoPE = R-matmul (pair swap w/ signs) + two
    elementwise multiplies with cos/sin tables (host-precomputed inputs).
  - scores computed TRANSPOSED (k on partitions) so no P transpose is
    needed; both blocks of a pair share one PSUM bank (cols 0:256 /
    256:512) so exp / 1/Z / normalize run as single [*, 512] ops; softmax
    needs no max-subtraction (scores ~N(0,1)); softmax denominators arrive
    replicated on PV-output partitions 64:128 via v_aug = [v_h | 1 x64];
    1/Z computed as exp(-ln Z) on ScalarE -- Ln and Exp share one
    activation-table set so no ACT_TABLE_LOAD churn; normalization folds
    into the attn PSUM->SBUF evacuation.
  - compute dtype bf16 (weights/x^T/q/k/P/v/attn), fp32 PSUM accumulate.
All scaling (1/sqrt(D)) and biases fold into host-prepped weights
(bq,bk folded on evac; bv,bo folded as bo_eff = bo + bv @ Wo since
softmax rows sum to one).
"""

import ml_dtypes
import numpy as np

import concourse.bass as bass
import concourse.tile as tile
from concourse import mybir
from concourse.bass_utils import run_bass_kernel_spmd

# ---------------------------------------------------------------- constants
B, NB, BS, F = 4, 16, 256, 1024
H, D, ROPE = 16, 64, 32
NCORES = 8
BLKS = B * NB                 # 64 blocks total
BPC = BLKS // NCORES          # 8 blocks per core
NPAIR = BPC // 2              # block pairs per core
BT = 2 * BS                   # tokens per pair (512)
F32 = mybir.dt.float32
BF16 = mybir.dt.bfloat16
WDT = BF16
WNP = ml_dtypes.bfloat16
MULT = mybir.AluOpType.mult
ADD = mybir.AluOpType.add
EXP = mybir.ActivationFunctionType.Exp
LN = mybir.ActivationFunctionType.Ln
IDENT_FN = mybir.ActivationFunctionType.Identity

# ------------------------------------------------- walrus multi-wait splitter
# This walrus build rejects >1 sync-wait per instruction on several
# instruction structs. Tile attaches several waits to one instruction;
# hoist extras onto NOPs inserted just before it on the same engine.
_split_ctr = [0]


def _split_multi_waits(nc, maxw=1):
    for f in nc.m.functions:
        for bb in f.blocks:
            insts = list(bb.instructions)
            out = []
            changed = False
            for inst in insts:
                si = inst.sync_info
                waits = list(si.on_wait) if si and si.on_wait else []
                if len(waits) > maxw:
                    changed = True
                    for w in waits[:-maxw]:
                        _split_ctr[0] += 1
                        nop = mybir.InstNoOp(
                            name=f"wsplit-{_split_ctr[0]}",
                            ins=[],
                            outs=[],
                            engine=inst.engine,
                        )
                        nop.sync_info = mybir.SyncInfo(on_wait=[w], on_update=[])
                        nc.register_instruction(nop)
                        out.append(nop)
                    si.on_wait = waits[-maxw:]
                out.append(inst)
            if changed:
                bb.instructions = out


# ---------------------------------------------------------------- bass build
class PairEmitter:
    """Emits one pair's work as phase closures so the caller can interleave
    pair p's attention chunks between pair p+1's projection phases."""

    def __init__(self, nc, pools, consts, pair, dram):
        self.nc = nc
        (self.psum, self.xin, self.xt, self.qk, self.ptp, self.attnp,
         self.outp, self.tabp) = pools
        (self.wq_sb, self.wk_sb, self.wv_sb, self.wo_sb, self.rt_sb,
         self.ident, self.bq_sb, self.bk_sb, self.bo_sb, self.vaug) = consts
        self.pair = pair
        (self.xq_d, self.xkv_d, self.cos_d, self.sin_d, self.out_d) = dram
        self.vaug_p = self.vaug[pair % 2]
        self.st = {}

    # ---- projection phases -------------------------------------------------
    def _transpose_input(self, x_d):
        nc = self.nc
        xt_in = {}
        for t in range(4):          # 4 token-chunks of 128
            for fh in range(2):
                xtile = self.xin.tile([128, 512], WDT, tag="xin",
                                      name=f"xin{t}{fh}")
                nc.gpsimd.dma_start(
                    out=xtile,
                    in_=x_d[
                        2 * self.pair + t // 2,
                        (t % 2) * 128 : (t % 2 + 1) * 128,
                        fh * 512 : (fh + 1) * 512,
                    ],
                )
                xt_in[(t, fh)] = xtile
        tiles = []
        for c in range(8):
            ps = self.psum.tile([128, BT], F32, tag="ps", bufs=5)
            for t in range(4):
                srct = xt_in[(t, c // 4)][:, (c % 4) * 128 : (c % 4 + 1) * 128]
                nc.tensor.matmul(
                    ps[:, t * 128 : (t + 1) * 128],
                    lhsT=srct,
                    rhs=self.ident,
                    start=True,
                    stop=True,
                )
            tt = self.xt.tile([128, BT], WDT, tag=f"xt{c}")
            nc.vector.tensor_copy(out=tt, in_=ps)
            tiles.append(tt)
        return tiles

    def _qk_proj(self, w_sb, b_sb, x_tiles, tagpfx):
        nc = self.nc
        cos_sb, sin_sb = self.st["cos"], self.st["sin"]
        outs = []
        for oc in range(8):
            ps = self.psum.tile([128, BT], F32, tag="ps", bufs=5)
            for c in range(8):
                nc.tensor.matmul(
                    ps,
                    lhsT=w_sb[c][:, oc * 128 : (oc + 1) * 128],
                    rhs=x_tiles[c],
                    start=(c == 0),
                    stop=(c == 7),
                )
            qf = self.qk.tile([128, BT], WDT, tag=f"{tagpfx}{oc}")
            if oc < 4:
                raw = self.qk.tile([128, BT], WDT, tag="raw", bufs=2)
                nc.vector.tensor_scalar_add(raw, ps, b_sb[:, oc : oc + 1])
                ps2 = self.psum.tile([128, BT], F32, tag="ps", bufs=5)
                nc.tensor.matmul(ps2, lhsT=self.rt_sb, rhs=raw,
                                 start=True, stop=True)
                qs2 = self.qk.tile([128, BT], WDT, tag="qs2", bufs=2)
                nc.vector.tensor_tensor(out=qs2, in0=ps2, in1=sin_sb, op=MULT)
                nc.gpsimd.tensor_tensor(out=qf, in0=raw, in1=cos_sb, op=MULT)
                nc.gpsimd.tensor_tensor(out=qf, in0=qf, in1=qs2, op=ADD)
            else:
                nc.vector.tensor_scalar_add(qf, ps, b_sb[:, oc : oc + 1])
            outs.append(qf)
        return outs

    def ph_tables_xq(self):
        nc = self.nc
        # gpsimd (SWDGE) queue: sync's queue head-of-line blocks behind the
        # previous pair's output stores, and rope needs these tables early.
        cos_sb = self.tabp.tile([128, BT], WDT, tag="cos", bufs=2)
        nc.gpsimd.dma_start(out=cos_sb, in_=self.cos_d[self.pair])
        sin_sb = self.tabp.tile([128, BT], WDT, tag="sin", bufs=2)
        nc.gpsimd.dma_start(out=sin_sb, in_=self.sin_d[self.pair])
        self.st["cos"], self.st["sin"] = cos_sb, sin_sb
        self.st["xqT"] = self._transpose_input(self.xq_d)

    def ph_qproj(self):
        self.st["qT"] = self._qk_proj(self.wq_sb, self.bq_sb,
                                      self.st["xqT"], "q")

    def ph_xkv(self):
        self.st["xkT"] = self._transpose_input(self.xkv_d)

    def ph_kproj(self):
        self.st["kT"] = self._qk_proj(self.wk_sb, self.bk_sb,
                                      self.st["xkT"], "k")

    def ph_vproj(self):
        # V projection into interleaved v_aug = [v_h | 1 x64] (128 cols/head).
        # The 64 ones-columns replicate the softmax row-sum onto PV output
        # partitions 64..127, already partition-broadcast for normalization.
        nc = self.nc
        xkT = self.st["xkT"]
        for kc in range(4):
            va = self.vaug_p[kc]
            va3 = va.rearrange("p (h c) -> p h c", c=128)
            for b2 in range(2):
                ps = self.psum.tile([128, 512], F32, tag="ps", bufs=5)
                for c in range(8):
                    nc.tensor.matmul(
                        ps,
                        lhsT=xkT[c][:, kc * 128 : (kc + 1) * 128],
                        rhs=self.wv_sb[c][:, b2 * 512 : (b2 + 1) * 512],
                        start=(c == 0),
                        stop=(c == 7),
                    )
                nc.vector.tensor_copy(
                    out=va3[:, b2 * 8 : (b2 + 1) * 8, 0:64],
                    in_=ps.rearrange("p (h c) -> p h c", c=64),
                )
        self.st["attnT"] = [
            self.attnp.tile([128, BT], WDT, tag=f"attnT{cc}",
                            name=f"attnT{cc}", bufs=1)
            for cc in range(8)
        ]

    def proj_phases(self):
        return [self.ph_tables_xq, self.ph_qproj, self.ph_xkv,
                self.ph_kproj, self.ph_vproj]

    # ---- attention chunks (per head-group) + output projection -------------
    def attn_scores(self, hg):
        """scores+exp for heads 4hg..4hg+3, both blocks packed into
        [128, 512] tiles (cols 0:256 = block 0, 256:512 = block 1)."""
        nc = self.nc
        qT, kT = self.st["qT"], self.st["kT"]
        rc, pc = hg, 4 + hg
        pts = {}
        for kc in range(2):
            sps = []
            for g in range(4):
                ps = self.psum.tile([128, BT], F32, tag="ps_sc", bufs=3)
                r0 = 32 * g
                for qh in range(2):
                    ksl = slice((qh * 2 + kc) * 128, (qh * 2 + kc + 1) * 128)
                    qsl = slice(qh * 256, (qh + 1) * 256)
                    nc.tensor.matmul(
                        ps[:, qsl],
                        lhsT=kT[rc][r0 : r0 + 32, ksl],
                        rhs=qT[rc][r0 : r0 + 32, qsl],
                        start=True,
                        stop=False,
                        tile_position=(r0, 0),
                    )
                    nc.tensor.matmul(
                        ps[:, qsl],
                        lhsT=kT[pc][r0 : r0 + 32, ksl],
                        rhs=qT[pc][r0 : r0 + 32, qsl],
                        start=False,
                        stop=True,
                        tile_position=(r0, 0),
                    )
                sps.append(ps)
            for g in range(4):
                pt = self.ptp.tile(
                    [128, BT], WDT, tag=f"pt{g}_{kc}", name=f"pt{g}_{kc}",
                    bufs=2,
                )
                nc.scalar.activation(out=pt, in_=sps[g], func=EXP)
                pts[(g, kc)] = pt
        self.st[f"pts{hg}"] = pts

    def attn_pv(self, hg):
        """PV matmuls; PSUM evacuated immediately to SBUF (DVE cast) so the
        bank never waits on the ScalarE 1/Z chain."""
        nc = self.nc
        pts = self.st.pop(f"pts{hg}")
        pvts = []
        for g in range(4):
            h = 4 * hg + g
            aps = self.psum.tile([128, BT], F32, tag="ps", bufs=5)
            for qh in range(2):
                qsl = slice(qh * 256, (qh + 1) * 256)
                for kc in range(2):
                    nc.tensor.matmul(
                        aps[:, qsl],
                        lhsT=self.vaug_p[qh * 2 + kc][:, h * 128 : (h + 1) * 128],
                        rhs=pts[(g, kc)][:, qsl],
                        start=(kc == 0),
                        stop=(kc == 1),
                    )
            pvt = self.attnp.tile([128, BT], WDT, tag=f"pvt{g}",
                                  name=f"pvt{g}", bufs=2)
            nc.vector.tensor_copy(out=pvt, in_=aps)
            pvts.append(pvt)
        self.st[f"pvts{hg}"] = pvts

    def attn_norm(self, hg):
        """1/Z (= exp(-ln Z), same act-table set) + normalized write into
        attnT; runs well after attn_pv so nothing here stalls the PE."""
        nc = self.nc
        attnT = self.st["attnT"]
        pvts = self.st.pop(f"pvts{hg}")
        for g in range(4):
            h = 4 * hg + g
            pvt = pvts[g]
            lnz = self.attnp.tile([64, BT], F32, tag="lnz", bufs=2)
            nc.scalar.activation(out=lnz, in_=pvt[64:128, :], func=LN)
            rec = self.attnp.tile([64, BT], WDT, tag="recip", bufs=2)
            nc.scalar.activation(out=rec, in_=lnz, func=EXP, scale=-1.0)
            cc, r0 = h // 2, (h % 2) * 64
            nc.vector.tensor_tensor(
                out=attnT[cc][r0 : r0 + 64, :],
                in0=pvt[0:64, :],
                in1=rec,
                op=MULT,
            )

    def out_proj(self):
        nc = self.nc
        attnT = self.st["attnT"]
        for t2 in range(4):
            for n2 in range(2):
                ps = self.psum.tile([128, 512], F32, tag="ps", bufs=5)
                for cc in range(8):
                    nc.tensor.matmul(
                        ps,
                        lhsT=attnT[cc][:, t2 * 128 : (t2 + 1) * 128],
                        rhs=self.wo_sb[cc][:, n2 * 512 : (n2 + 1) * 512],
                        start=(cc == 0),
                        stop=(cc == 7),
                    )
                ob = self.outp.tile([128, 512], F32, tag="outsb")
                nc.vector.tensor_tensor(
                    out=ob,
                    in0=ps,
                    in1=self.bo_sb[:, n2 * 512 : (n2 + 1) * 512],
                    op=ADD,
                )
                nc.sync.dma_start(
                    out=self.out_d[
                        2 * self.pair + t2 // 2,
                        (t2 % 2) * 128 : (t2 % 2 + 1) * 128,
                        n2 * 512 : (n2 + 1) * 512,
                    ],
                    in_=ob,
                )

    def tail_chunks(self):
        A = [lambda hg=hg: self.attn_scores(hg) for hg in range(4)]
        P = [lambda hg=hg: self.attn_pv(hg) for hg in range(4)]
        L = [lambda hg=hg: self.attn_norm(hg) for hg in range(4)]
        return [A[0], P[0], A[1], L[0], P[1], A[2], L[1], P[2], A[3],
                L[2], P[3], L[3], self.out_proj]

def build_program():
    nc = bass.Bass("TRN2")
    xq_d = nc.dram_tensor("xq", [BPC, BS, F], F32, kind="ExternalInput")
    xkv_d = nc.dram_tensor("xkv", [BPC, BS, F], F32, kind="ExternalInput")
    wq_d = nc.dram_tensor("wq", [8, 128, F], WDT, kind="ExternalInput")
    wk_d = nc.dram_tensor("wk", [8, 128, F], WDT, kind="ExternalInput")
    wv_d = nc.dram_tensor("wv", [8, 128, F], WDT, kind="ExternalInput")
    wo_d = nc.dram_tensor("wo", [8, 128, F], WDT, kind="ExternalInput")
    rt_d = nc.dram_tensor("rt", [128, 128], WDT, kind="ExternalInput")
    ident_d = nc.dram_tensor("ident", [128, 128], WDT, kind="ExternalInput")
    ones_d = nc.dram_tensor("ones", [1, 16, 64], WDT, kind="ExternalInput")
    bq_d = nc.dram_tensor("bq", [128, 8], F32, kind="ExternalInput")
    bk_d = nc.dram_tensor("bk", [128, 8], F32, kind="ExternalInput")
    bo_d = nc.dram_tensor("bo", [1, F], F32, kind="ExternalInput")
    cos_d = nc.dram_tensor("cos", [NPAIR, 128, BT], WDT, kind="ExternalInput")
    sin_d = nc.dram_tensor("sin", [NPAIR, 128, BT], WDT, kind="ExternalInput")
    out_d = nc.dram_tensor("out", [BPC, BS, F], F32, kind="ExternalOutput")

    with tile.TileContext(nc) as tc:
        with (
            tc.tile_pool(name="wpool", bufs=1) as wpool,
            tc.tile_pool(name="psum", bufs=8, space="PSUM") as psum,
            tc.tile_pool(name="xin", bufs=4) as xin,
            tc.tile_pool(name="xt", bufs=2) as xt,
            tc.tile_pool(name="qk", bufs=2) as qk,
            tc.tile_pool(name="ptp", bufs=1) as ptp,
            tc.tile_pool(name="attnp", bufs=1) as attnp,
            tc.tile_pool(name="outp", bufs=2) as outp,
            tc.tile_pool(name="tabp", bufs=1) as tabp,
        ):
            def wtiles(src, tagpfx):
                ts = []
                for c in range(8):
                    t = wpool.tile([128, F], WDT, tag=f"{tagpfx}{c}", name=f"{tagpfx}{c}")
                    nc.sync.dma_start(out=t, in_=src[c])
                    ts.append(t)
                return ts

            # load order matters on the sync queue: first pair needs ident
            # (transposes), wq, rope consts before anything else.
            ident = wpool.tile([128, 128], WDT, tag="ident")
            nc.sync.dma_start(out=ident, in_=ident_d[:])
            rt_sb = wpool.tile([128, 128], WDT, tag="rt")
            nc.sync.dma_start(out=rt_sb, in_=rt_d[:])
            bq_sb = wpool.tile([128, 8], F32, tag="bq")
            nc.sync.dma_start(out=bq_sb, in_=bq_d[:])
            bk_sb = wpool.tile([128, 8], F32, tag="bk")
            nc.sync.dma_start(out=bk_sb, in_=bk_d[:])
            wq_sb = wtiles(wq_d, "wq")
            wk_sb = wtiles(wk_d, "wk")
            wv_sb = wtiles(wv_d, "wv")
            wo_sb = wtiles(wo_d, "wo")
            bo_sb = wpool.tile([128, F], F32, tag="bo")
            nc.sync.dma_start(out=bo_sb, in_=bo_d[0:1, :].to_broadcast([128, F]))

            vaug = []
            for par in range(2):
                vset = []
                for kc in range(4):
                    va = wpool.tile(
                        [128, 2048], WDT,
                        tag=f"vaug{par}{kc}", name=f"vaug{par}{kc}",
                    )
                    nc.sync.dma_start(
                        out=va.rearrange("p (h c) -> p h c", c=128)[:, :, 64:128],
                        in_=ones_d[:].to_broadcast([128, 16, 64]),
                    )
                    vset.append(va)
                vaug.append(vset)

            pools = (psum, xin, xt, qk, ptp, attnp, outp, tabp)
            consts = (
                wq_sb, wk_sb, wv_sb, wo_sb, rt_sb, ident, bq_sb, bk_sb, bo_sb, vaug
            )
            dram = (xq_d, xkv_d, cos_d, sin_d, out_d)

            # software pipeline: pair p's attention/output chunks emitted
            # between pair p+1's projection phases.
            # distribute the 13 tail chunks of pair p-1 over pair p's 5
            # projection phases: scores / PV / normalize stages each land a
            # full phase after their producers.
            bounds = [1, 3, 6, 9, 13]
            prev_tail = []
            for pair in range(NPAIR):
                em = PairEmitter(nc, pools, consts, pair, dram)
                phases = em.proj_phases()
                done = 0
                for i, ph in enumerate(phases):
                    ph()
                    upto = min(bounds[i], len(prev_tail))
                    for c in prev_tail[done:upto]:
                        c()
                    done = upto
                for c in prev_tail[done:]:
                    c()
                prev_tail = em.tail_chunks()
            for c in prev_tail:
                c()

    _split_multi_waits(nc)
    return nc


# ---------------------------------------------------------------- host side
def _host_prep(Wq, bq, Wk, bk, Wv, bv, Wo, bo):
    """Permute/scale weights; fold biases."""
    old_of_new = np.empty(F, np.int64)
    for h in range(H):
        old_of_new[h * ROPE : (h + 1) * ROPE] = h * D + np.arange(ROPE)
        old_of_new[512 + h * ROPE : 512 + (h + 1) * ROPE] = (
            h * D + ROPE + np.arange(ROPE)
        )
    wq_flat = (Wq.reshape(F, F) / np.sqrt(D)).astype(np.float32)
    wq_p = np.ascontiguousarray(wq_flat[:, old_of_new]).reshape(8, 128, F)
    wk_flat = Wk.reshape(F, F).astype(np.float32)
    wk_p = np.ascontiguousarray(wk_flat[:, old_of_new]).reshape(8, 128, F)
    wv_c = np.ascontiguousarray(Wv.reshape(F, F)).reshape(8, 128, F)
    wo_c = np.ascontiguousarray(Wo.reshape(F, F)).reshape(8, 128, F)
    bq_p = np.ascontiguousarray(
        (bq.reshape(F) / np.sqrt(D))[old_of_new].reshape(8, 128).T
    ).astype(np.float32)
    bk_p = np.ascontiguousarray(bk.reshape(F)[old_of_new].reshape(8, 128).T).astype(
        np.float32
    )
    bo_eff = (bo + bv.reshape(F) @ Wo.reshape(F, F)).reshape(1, F).astype(np.float32)

    # R^T for rotate_every_two with signs: (R@q)[2i] = -q[2i+1]; [2i+1] = q[2i]
    R = np.zeros((128, 128), np.float32)
    for g in range(4):          # 4 heads per rope chunk, 32 rows each
        for i in range(ROPE // 2):
            R[g * 32 + 2 * i, g * 32 + 2 * i + 1] = -1.0
            R[g * 32 + 2 * i + 1, g * 32 + 2 * i] = 1.0
    rt = np.ascontiguousarray(R.T)
    return wq_p, wk_p, wv_c, wo_c, bq_p, bk_p, bo_eff, rt


def _tables_for_core(core):
    """cos/sin tables [NPAIR, 128, 512] for this core's block pairs."""
    inv_freq = 1.0 / 10000.0 ** (np.arange(0, ROPE, 2) / ROPE)
    cos_t = np.empty((NPAIR, 128, BT), np.float32)
    sin_t = np.empty((NPAIR, 128, BT), np.float32)
    for p in range(NPAIR):
        for half in range(2):
            nb = (core * BPC + 2 * p + half) % NB
            pos = nb * BS + np.arange(BS, dtype=np.float64)
            ang = pos[None, :] * inv_freq[:, None]          # [16, 256]
            cpat = np.repeat(np.cos(ang), 2, axis=0)        # [32, 256]
            spat = np.repeat(np.sin(ang), 2, axis=0)
            sl = slice(half * BS, (half + 1) * BS)
            cos_t[p, :, sl] = np.tile(cpat, (4, 1))
            sin_t[p, :, sl] = np.tile(spat, (4, 1))
    return cos_t, sin_t


_nc_cache = []


def kernel(inputs_q, inputs_kv, Wq, bq, Wk, bk, Wv, bv, Wo, bo):
    inputs_q = np.asarray(inputs_q, np.float32)
    inputs_kv = np.asarray(inputs_kv, np.float32)
    wq_p, wk_p, wv_c, wo_c, bq_p, bk_p, bo_eff, rt = _host_prep(
        np.asarray(Wq), np.asarray(bq), np.asarray(Wk), np.asarray(bk),
        np.asarray(Wv), np.asarray(bv), np.asarray(Wo), np.asarray(bo),
    )
    xq_all = inputs_q.reshape(BLKS, BS, F)
    xkv_all = inputs_kv.reshape(BLKS, BS, F)
    wq_p = wq_p.astype(WNP)
    wk_p = wk_p.astype(WNP)
    wv_c = wv_c.astype(WNP)
    wo_c = wo_c.astype(WNP)

    if not _nc_cache:
        _nc_cache.append(build_program())
    nc = _nc_cache[0]

    in_maps = []
    for core in range(NCORES):
        cos_t, sin_t = _tables_for_core(core)
        in_maps.append(
            {
                "xq": np.ascontiguousarray(xq_all[core * BPC : (core + 1) * BPC]),
                "xkv": np.ascontiguousarray(xkv_all[core * BPC : (core + 1) * BPC]),
                "wq": wq_p, "wk": wk_p, "wv": wv_c, "wo": wo_c,
                "rt": rt.astype(WNP), "bq": bq_p, "bk": bk_p, "bo": bo_eff,
                "ident": np.eye(128, dtype=WNP),
                "ones": np.ones((1, 16, 64), WNP),
                "cos": cos_t.astype(WNP), "sin": sin_t.astype(WNP),
            }
        )
    res = run_bass_kernel_spmd(nc, in_maps, list(range(NCORES)))
    out = np.concatenate([res.results[i]["out"] for i in range(NCORES)], axis=0)
    return out.reshape(B, NB, BS, F)


# revision 16
# speedup vs baseline: 1.0542x; 1.0542x over previous
"""Trainium2 Bass kernel for nn_MultiHeadDotProductAttention_14980845928960.

Block-local multi-head attention with partial RoPE:
  q/k/v projections -> RoPE on first 32 of 64 head dims -> softmax(QK^T/8)V
  -> output projection.  Shapes: inputs [4,16,256,1024], 16 heads x 64 dim,
  blocks of 256 tokens attend locally.

Strategy: data-parallel over the 64 (batch, block) pairs -> 8 blocks/core.
Projections are batched over PAIRS of blocks (512 tokens -> N=512 moving
operands); attention runs per 256-token block but with both blocks of a
pair packed into [128, 512] tiles.  The emission order SOFTWARE-PIPELINES
pairs: pair p's ScalarE-heavy attention chunks are interleaved between
pair p+1's PE-heavy projection phases, so the tensor engine never idles
long enough for the HAM clock gate to re-throttle it.
Everything keeps the contraction dim on SBUF partitions:
  - inputs DMA'd with fp32->bf16 cast (gpsimd SWDGE); x^T via REGULAR bf16
    matmuls against a bf16 identity (lhsT = x chunk) -- all-bf16 PE keeps
    fast-weight-load enabled; projections as lhsT=W chunk, rhs=x^T chunk.
  - Q/K channel-PERMUTED (host side) so rope dims occupy out-chunks 0-3
    and pass dims chunks 4-7; RoPE = R-matmul (pair swap w/ signs) + two
    elementwise multiplies with cos/sin tables (host-precomputed inputs).
  - scores computed TRANSPOSED (k on partitions) so no P transpose is
    needed; both blocks of a pair share one PSUM bank (cols 0:256 /
    256:512) so exp / 1/Z / normalize run as single [*, 512] ops; softmax
    needs no max-subtraction (scores ~N(0,1)); softmax denominators arrive
    replicated on PV-output partitions 64:128 via v_aug = [v_h | 1 x64];
    1/Z computed as exp(-ln Z) on ScalarE -- Ln and Exp share one
    activation-table set so no ACT_TABLE_LOAD churn; normalization folds
    into the attn PSUM->SBUF evacuation.
  - compute dtype bf16 (weights/x^T/q/k/P/v/attn), fp32 PSUM accumulate.
All scaling (1/sqrt(D)) and biases fold into host-prepped weights
(bq,bk folded on evac; bv,bo folded as bo_eff = bo + bv @ Wo since
softmax rows sum to one).
"""

import ml_dtypes
import numpy as np

import concourse.bass as bass
import concourse.tile as tile
from concourse import mybir
from concourse.bass_utils import run_bass_kernel_spmd

# ---------------------------------------------------------------- constants
B, NB, BS, F = 4, 16, 256, 1024
H, D, ROPE = 16, 64, 32
NCORES = 8
BLKS = B * NB                 # 64 blocks total
BPC = BLKS // NCORES          # 8 blocks per core
NPAIR = BPC // 2              # block pairs per core
BT = 2 * BS                   # tokens per pair (512)
F32 = mybir.dt.float32
BF16 = mybir.dt.bfloat16
WDT = BF16
WNP = ml_dtypes.bfloat16
MULT = mybir.AluOpType.mult
ADD = mybir.AluOpType.add
EXP = mybir.ActivationFunctionType.Exp
LN = mybir.ActivationFunctionType.Ln
IDENT_FN = mybir.ActivationFunctionType.Identity

# ------------------------------------------------- walrus multi-wait splitter
# This walrus build rejects >1 sync-wait per instruction on several
# instruction structs. Tile attaches several waits to one instruction;
# hoist extras onto NOPs inserted just before it on the same engine.
_split_ctr = [0]


def _split_multi_waits(nc, maxw=1):
    for f in nc.m.functions:
        for bb in f.blocks:
            insts = list(bb.instructions)
            out = []
            changed = False
            for inst in insts:
                si = inst.sync_info
                waits = list(si.on_wait) if si and si.on_wait else []
                if len(waits) > maxw:
                    changed = True
                    for w in waits[:-maxw]:
                        _split_ctr[0] += 1
                        nop = mybir.InstNoOp(
                            name=f"wsplit-{_split_ctr[0]}",
                            ins=[],
                            outs=[],
                            engine=inst.engine,
                        )
                        nop.sync_info = mybir.SyncInfo(on_wait=[w], on_update=[])
                        nc.register_instruction(nop)
                        out.append(nop)
                    si.on_wait = waits[-maxw:]
                out.append(inst)
            if changed:
                bb.instructions = out


# ---------------------------------------------------------------- bass build
class PairEmitter:
    """Emits one pair's work as phase closures so the caller can interleave
    pair p's attention chunks between pair p+1's projection phases."""

    def __init__(self, nc, pools, consts, pair, dram):
        self.nc = nc
        (self.psum, self.xin, self.xt, self.qk, self.ptp, self.attnp,
         self.outp, self.tabp) = pools
        (self.wq_sb, self.wk_sb, self.wv_sb, self.wo_sb, self.rt_sb,
         self.ident, self.bq_sb, self.bk_sb, self.bo_sb, self.vaug) = consts
        self.pair = pair
        (self.xq_d, self.xkv_d, self.cos_d, self.sin_d, self.out_d) = dram
        self.vaug_p = self.vaug[pair % 2]
        self.st = {}

    # ---- projection phases -------------------------------------------------
    def _transpose_input(self, x_d):
        nc = self.nc
        xt_in = {}
        for t in range(4):          # 4 token-chunks of 128
            for fh in range(2):
                xtile = self.xin.tile([128, 512], WDT, tag="xin",
                                      name=f"xin{t}{fh}")
                nc.gpsimd.dma_start(
                    out=xtile,
                    in_=x_d[
                        2 * self.pair + t // 2,
                        (t % 2) * 128 : (t % 2 + 1) * 128,
                        fh * 512 : (fh + 1) * 512,
                    ],
                )
                xt_in[(t, fh)] = xtile
        tiles = []
        for c in range(8):
            ps = self.psum.tile([128, BT], F32, tag="ps", bufs=4)
            for t in range(4):
                srct = xt_in[(t, c // 4)][:, (c % 4) * 128 : (c % 4 + 1) * 128]
                nc.tensor.matmul(
                    ps[:, t * 128 : (t + 1) * 128],
                    lhsT=srct,
                    rhs=self.ident,
                    start=True,
                    stop=True,
                )
            tt = self.xt.tile([128, BT], WDT, tag=f"xt{c}")
            nc.vector.tensor_copy(out=tt, in_=ps)
            tiles.append(tt)
        return tiles

    def _qk_proj(self, w_sb, b_sb, x_tiles, tagpfx):
        nc = self.nc
        cos_sb, sin_sb = self.st["cos"], self.st["sin"]
        outs = []
        for oc in range(8):
            ps = self.psum.tile([128, BT], F32, tag="ps", bufs=4)
            for c in range(8):
                nc.tensor.matmul(
                    ps,
                    lhsT=w_sb[c][:, oc * 128 : (oc + 1) * 128],
                    rhs=x_tiles[c],
                    start=(c == 0),
                    stop=(c == 7),
                )
            qf = self.qk.tile([128, BT], WDT, tag=f"{tagpfx}{oc}")
            if oc < 4:
                raw = self.qk.tile([128, BT], WDT, tag="raw", bufs=2)
                nc.vector.tensor_scalar_add(raw, ps, b_sb[:, oc : oc + 1])
                ps2 = self.psum.tile([128, BT], F32, tag="ps", bufs=4)
                nc.tensor.matmul(ps2, lhsT=self.rt_sb, rhs=raw,
                                 start=True, stop=True)
                qs2 = self.qk.tile([128, BT], WDT, tag="qs2", bufs=2)
                nc.vector.tensor_tensor(out=qs2, in0=ps2, in1=sin_sb, op=MULT)
                nc.gpsimd.tensor_tensor(out=qf, in0=raw, in1=cos_sb, op=MULT)
                nc.gpsimd.tensor_tensor(out=qf, in0=qf, in1=qs2, op=ADD)
            else:
                nc.vector.tensor_scalar_add(qf, ps, b_sb[:, oc : oc + 1])
            outs.append(qf)
        return outs

    def ph_tables_xq(self):
        nc = self.nc
        # gpsimd (SWDGE) queue: sync's queue head-of-line blocks behind the
        # previous pair's output stores, and rope needs these tables early.
        cos_sb = self.tabp.tile([128, BT], WDT, tag="cos", bufs=2)
        nc.gpsimd.dma_start(out=cos_sb, in_=self.cos_d[self.pair])
        sin_sb = self.tabp.tile([128, BT], WDT, tag="sin", bufs=2)
        nc.gpsimd.dma_start(out=sin_sb, in_=self.sin_d[self.pair])
        self.st["cos"], self.st["sin"] = cos_sb, sin_sb
        self.st["xqT"] = self._transpose_input(self.xq_d)

    def ph_qproj(self):
        self.st["qT"] = self._qk_proj(self.wq_sb, self.bq_sb,
                                      self.st["xqT"], "q")

    def ph_xkv(self):
        self.st["xkT"] = self._transpose_input(self.xkv_d)

    def ph_kproj(self):
        self.st["kT"] = self._qk_proj(self.wk_sb, self.bk_sb,
                                      self.st["xkT"], "k")

    def ph_vproj(self):
        # V projection into interleaved v_aug = [v_h | 1 x64] (128 cols/head).
        # The 64 ones-columns replicate the softmax row-sum onto PV output
        # partitions 64..127, already partition-broadcast for normalization.
        nc = self.nc
        xkT = self.st["xkT"]
        for kc in range(4):
            va = self.vaug_p[kc]
            va3 = va.rearrange("p (h c) -> p h c", c=128)
            for b2 in range(2):
                ps = self.psum.tile([128, 512], F32, tag="ps", bufs=4)
                for c in range(8):
                    nc.tensor.matmul(
                        ps,
                        lhsT=xkT[c][:, kc * 128 : (kc + 1) * 128],
                        rhs=self.wv_sb[c][:, b2 * 512 : (b2 + 1) * 512],
                        start=(c == 0),
                        stop=(c == 7),
                    )
                nc.scalar.activation(
                    out=va3[:, b2 * 8 : (b2 + 1) * 8, 0:64],
                    in_=ps.rearrange("p (h c) -> p h c", c=64),
                    func=IDENT_FN,
                )
        self.st["attnT"] = [
            self.attnp.tile([128, BT], WDT, tag=f"attnT{cc}",
                            name=f"attnT{cc}", bufs=2)
            for cc in range(8)
        ]

    def proj_phases(self):
        return [self.ph_tables_xq, self.ph_qproj, self.ph_xkv,
                self.ph_kproj, self.ph_vproj]

    # ---- attention chunks (per head-group) + output projection -------------
    def attn_hg(self, hg):
        """scores+exp+PV+normalize for heads 4hg..4hg+3, both blocks packed
        into [*, 512] col-groups; PSUM tiles span TWO banks ([128,1024] f32,
        one matmul per bank-half) so each ScalarE activation covers two
        heads."""
        nc = self.nc
        qT, kT = self.st["qT"], self.st["kT"]
        attnT = self.st["attnT"]
        rc, pc = hg, 4 + hg
        pts = {}
        for kc in range(2):
            for gp in range(2):     # head-pairs (g = 2gp, 2gp+1)
                ps = self.psum.tile([128, 2 * BT], F32, tag="ps_att", bufs=2)
                for gi in range(2):
                    g = 2 * gp + gi
                    r0 = 32 * g
                    for qh in range(2):
                        ksl = slice((qh * 2 + kc) * 128, (qh * 2 + kc + 1) * 128)
                        qsl = slice(gi * BT + qh * 256, gi * BT + (qh + 1) * 256)
                        qsl_q = slice(qh * 256, (qh + 1) * 256)
                        nc.tensor.matmul(
                            ps[:, qsl],
                            lhsT=kT[rc][r0 : r0 + 32, ksl],
                            rhs=qT[rc][r0 : r0 + 32, qsl_q],
                            start=True,
                            stop=False,
                            tile_position=(r0, 0),
                        )
                        nc.tensor.matmul(
                            ps[:, qsl],
                            lhsT=kT[pc][r0 : r0 + 32, ksl],
                            rhs=qT[pc][r0 : r0 + 32, qsl_q],
                            start=False,
                            stop=True,
                            tile_position=(r0, 0),
                        )
                pt = self.ptp.tile(
                    [128, 2 * BT], WDT, tag=f"pt{gp}_{kc}", name=f"pt{gp}_{kc}",
                    bufs=2,
                )
                nc.scalar.activation(out=pt, in_=ps, func=EXP)
                for gi in range(2):
                    pts[(2 * gp + gi, kc)] = pt[:, gi * BT : (gi + 1) * BT]
        # PV (head-pair per two-bank tile) + 1/Z (= exp(-ln Z), same
        # act-table set, two heads per activation) + normalized evacuation
        for gp in range(2):
            aps = self.psum.tile([128, 2 * BT], F32, tag="ps_att", bufs=2)
            for gi in range(2):
                g = 2 * gp + gi
                h = 4 * hg + g
                for qh in range(2):
                    qsl = slice(gi * BT + qh * 256, gi * BT + (qh + 1) * 256)
                    qsl_q = slice(qh * 256, (qh + 1) * 256)
                    for kc in range(2):
                        nc.tensor.matmul(
                            aps[:, qsl],
                            lhsT=self.vaug_p[qh * 2 + kc][:, h * 128 : (h + 1) * 128],
                            rhs=pts[(g, kc)][:, qsl_q],
                            start=(kc == 0),
                            stop=(kc == 1),
                        )
            lnz = self.attnp.tile([64, 2 * BT], F32, tag="lnz", bufs=1)
            nc.scalar.activation(out=lnz, in_=aps[64:128, :], func=LN)
            rec = self.attnp.tile([64, 2 * BT], WDT, tag="recip", bufs=2)
            nc.scalar.activation(out=rec, in_=lnz, func=EXP, scale=-1.0)
            for gi in range(2):
                h = 4 * hg + 2 * gp + gi
                cc, r0 = h // 2, (h % 2) * 64
                nc.vector.tensor_tensor(
                    out=attnT[cc][r0 : r0 + 64, :],
                    in0=aps[0:64, gi * BT : (gi + 1) * BT],
                    in1=rec[:, gi * BT : (gi + 1) * BT],
                    op=MULT,
                )

    def out_proj(self):
        nc = self.nc
        attnT = self.st["attnT"]
        for t2 in range(4):
            for n2 in range(2):
                ps = self.psum.tile([128, 512], F32, tag="ps", bufs=4)
                for cc in range(8):
                    nc.tensor.matmul(
                        ps,
                        lhsT=attnT[cc][:, t2 * 128 : (t2 + 1) * 128],
                        rhs=self.wo_sb[cc][:, n2 * 512 : (n2 + 1) * 512],
                        start=(cc == 0),
                        stop=(cc == 7),
                    )
                ob = self.outp.tile([128, 512], F32, tag="outsb")
                nc.vector.tensor_tensor(
                    out=ob,
                    in0=ps,
                    in1=self.bo_sb[:, n2 * 512 : (n2 + 1) * 512],
                    op=ADD,
                )
                nc.sync.dma_start(
                    out=self.out_d[
                        2 * self.pair + t2 // 2,
                        (t2 % 2) * 128 : (t2 % 2 + 1) * 128,
                        n2 * 512 : (n2 + 1) * 512,
                    ],
                    in_=ob,
                )

    def tail_chunks(self):
        return [lambda hg=hg: self.attn_hg(hg) for hg in range(4)] + [
            self.out_proj
        ]


def build_program():
    nc = bass.Bass("TRN2")
    xq_d = nc.dram_tensor("xq", [BPC, BS, F], F32, kind="ExternalInput")
    xkv_d = nc.dram_tensor("xkv", [BPC, BS, F], F32, kind="ExternalInput")
    wq_d = nc.dram_tensor("wq", [8, 128, F], WDT, kind="ExternalInput")
    wk_d = nc.dram_tensor("wk", [8, 128, F], WDT, kind="ExternalInput")
    wv_d = nc.dram_tensor("wv", [8, 128, F], WDT, kind="ExternalInput")
    wo_d = nc.dram_tensor("wo", [8, 128, F], WDT, kind="ExternalInput")
    rt_d = nc.dram_tensor("rt", [128, 128], WDT, kind="ExternalInput")
    ident_d = nc.dram_tensor("ident", [128, 128], WDT, kind="ExternalInput")
    ones_d = nc.dram_tensor("ones", [1, 16, 64], WDT, kind="ExternalInput")
    bq_d = nc.dram_tensor("bq", [128, 8], F32, kind="ExternalInput")
    bk_d = nc.dram_tensor("bk", [128, 8], F32, kind="ExternalInput")
    bo_d = nc.dram_tensor("bo", [1, F], F32, kind="ExternalInput")
    cos_d = nc.dram_tensor("cos", [NPAIR, 128, BT], WDT, kind="ExternalInput")
    sin_d = nc.dram_tensor("sin", [NPAIR, 128, BT], WDT, kind="ExternalInput")
    out_d = nc.dram_tensor("out", [BPC, BS, F], F32, kind="ExternalOutput")

    with tile.TileContext(nc) as tc:
        with (
            tc.tile_pool(name="wpool", bufs=1) as wpool,
            tc.tile_pool(name="psum", bufs=8, space="PSUM") as psum,
            tc.tile_pool(name="xin", bufs=4) as xin,
            tc.tile_pool(name="xt", bufs=2) as xt,
            tc.tile_pool(name="qk", bufs=2) as qk,
            tc.tile_pool(name="ptp", bufs=1) as ptp,
            tc.tile_pool(name="attnp", bufs=1) as attnp,
            tc.tile_pool(name="outp", bufs=2) as outp,
            tc.tile_pool(name="tabp", bufs=1) as tabp,
        ):
            def wtiles(src, tagpfx):
                ts = []
                for c in range(8):
                    t = wpool.tile([128, F], WDT, tag=f"{tagpfx}{c}", name=f"{tagpfx}{c}")
                    nc.sync.dma_start(out=t, in_=src[c])
                    ts.append(t)
                return ts

            # load order matters on the sync queue: first pair needs ident
            # (transposes), wq, rope consts before anything else.
            ident = wpool.tile([128, 128], WDT, tag="ident")
            nc.sync.dma_start(out=ident, in_=ident_d[:])
            rt_sb = wpool.tile([128, 128], WDT, tag="rt")
            nc.sync.dma_start(out=rt_sb, in_=rt_d[:])
            bq_sb = wpool.tile([128, 8], F32, tag="bq")
            nc.sync.dma_start(out=bq_sb, in_=bq_d[:])
            bk_sb = wpool.tile([128, 8], F32, tag="bk")
            nc.sync.dma_start(out=bk_sb, in_=bk_d[:])
            wq_sb = wtiles(wq_d, "wq")
            wk_sb = wtiles(wk_d, "wk")
            wv_sb = wtiles(wv_d, "wv")
            wo_sb = wtiles(wo_d, "wo")
            bo_sb = wpool.tile([128, F], F32, tag="bo")
            nc.sync.dma_start(out=bo_sb, in_=bo_d[0:1, :].to_broadcast([128, F]))

            vaug = []
            for par in range(2):
                vset = []
                for kc in range(4):
                    va = wpool.tile(
                        [128, 2048], WDT,
                        tag=f"vaug{par}{kc}", name=f"vaug{par}{kc}",
                    )
                    nc.sync.dma_start(
                        out=va.rearrange("p (h c) -> p h c", c=128)[:, :, 64:128],
                        in_=ones_d[:].to_broadcast([128, 16, 64]),
                    )
                    vset.append(va)
                vaug.append(vset)

            pools = (psum, xin, xt, qk, ptp, attnp, outp, tabp)
            consts = (
                wq_sb, wk_sb, wv_sb, wo_sb, rt_sb, ident, bq_sb, bk_sb, bo_sb, vaug
            )
            dram = (xq_d, xkv_d, cos_d, sin_d, out_d)

            # software pipeline: pair p's attention/output chunks emitted
            # between pair p+1's projection phases.
            prev_tail = []
            for pair in range(NPAIR):
                em = PairEmitter(nc, pools, consts, pair, dram)
                phases = em.proj_phases()
                for i, ph in enumerate(phases):
                    ph()
                    if i < len(prev_tail):
                        prev_tail[i]()
                for c in prev_tail[len(phases):]:
                    c()
                prev_tail = em.tail_chunks()
            for c in prev_tail:
                c()

    _split_multi_waits(nc)
    return nc


# ---------------------------------------------------------------- host side
def _host_prep(Wq, bq, Wk, bk, Wv, bv, Wo, bo):
    """Permute/scale weights; fold biases."""
    old_of_new = np.empty(F, np.int64)
    for h in range(H):
        old_of_new[h * ROPE : (h + 1) * ROPE] = h * D + np.arange(ROPE)
        old_of_new[512 + h * ROPE : 512 + (h + 1) * ROPE] = (
            h * D + ROPE + np.arange(ROPE)
        )
    wq_flat = (Wq.reshape(F, F) / np.sqrt(D)).astype(np.float32)
    wq_p = np.ascontiguousarray(wq_flat[:, old_of_new]).reshape(8, 128, F)
    wk_flat = Wk.reshape(F, F).astype(np.float32)
    wk_p = np.ascontiguousarray(wk_flat[:, old_of_new]).reshape(8, 128, F)
    wv_c = np.ascontiguousarray(Wv.reshape(F, F)).reshape(8, 128, F)
    wo_c = np.ascontiguousarray(Wo.reshape(F, F)).reshape(8, 128, F)
    bq_p = np.ascontiguousarray(
        (bq.reshape(F) / np.sqrt(D))[old_of_new].reshape(8, 128).T
    ).astype(np.float32)
    bk_p = np.ascontiguousarray(bk.reshape(F)[old_of_new].reshape(8, 128).T).astype(
        np.float32
    )
    bo_eff = (bo + bv.reshape(F) @ Wo.reshape(F, F)).reshape(1, F).astype(np.float32)

    # R^T for rotate_every_two with signs: (R@q)[2i] = -q[2i+1]; [2i+1] = q[2i]
    R = np.zeros((128, 128), np.float32)
    for g in range(4):          # 4 heads per rope chunk, 32 rows each
        for i in range(ROPE // 2):
            R[g * 32 + 2 * i, g * 32 + 2 * i + 1] = -1.0
            R[g * 32 + 2 * i + 1, g * 32 + 2 * i] = 1.0
    rt = np.ascontiguousarray(R.T)
    return wq_p, wk_p, wv_c, wo_c, bq_p, bk_p, bo_eff, rt


def _tables_for_core(core):
    """cos/sin tables [NPAIR, 128, 512] for this core's block pairs."""
    inv_freq = 1.0 / 10000.0 ** (np.arange(0, ROPE, 2) / ROPE)
    cos_t = np.empty((NPAIR, 128, BT), np.float32)
    sin_t = np.empty((NPAIR, 128, BT), np.float32)
    for p in range(NPAIR):
        for half in range(2):
            nb = (core * BPC + 2 * p + half) % NB
            pos = nb * BS + np.arange(BS, dtype=np.float64)
            ang = pos[None, :] * inv_freq[:, None]          # [16, 256]
            cpat = np.repeat(np.cos(ang), 2, axis=0)        # [32, 256]
            spat = np.repeat(np.sin(ang), 2, axis=0)
            sl = slice(half * BS, (half + 1) * BS)
            cos_t[p, :, sl] = np.tile(cpat, (4, 1))
            sin_t[p, :, sl] = np.tile(spat, (4, 1))
    return cos_t, sin_t


_nc_cache = []


def kernel(inputs_q, inputs_kv, Wq, bq, Wk, bk, Wv, bv, Wo, bo):
    inputs_q = np.asarray(inputs_q, np.float32)
    inputs_kv = np.asarray(inputs_kv, np.float32)
    wq_p, wk_p, wv_c, wo_c, bq_p, bk_p, bo_eff, rt = _host_prep(
        np.asarray(Wq), np.asarray(bq), np.asarray(Wk), np.asarray(bk),
        np.asarray(Wv), np.asarray(bv), np.asarray(Wo), np.asarray(bo),
    )
    xq_all = inputs_q.reshape(BLKS, BS, F)
    xkv_all = inputs_kv.reshape(BLKS, BS, F)
    wq_p = wq_p.astype(WNP)
    wk_p = wk_p.astype(WNP)
    wv_c = wv_c.astype(WNP)
    wo_c = wo_c.astype(WNP)

    if not _nc_cache:
        _nc_cache.append(build_program())
    nc = _nc_cache[0]

    in_maps = []
    for core in range(NCORES):
        cos_t, sin_t = _tables_for_core(core)
        in_maps.append(
            {
                "xq": np.ascontiguousarray(xq_all[core * BPC : (core + 1) * BPC]),
                "xkv": np.ascontiguousarray(xkv_all[core * BPC : (core + 1) * BPC]),
                "wq": wq_p, "wk": wk_p, "wv": wv_c, "wo": wo_c,
                "rt": rt.astype(WNP), "bq": bq_p, "bk": bk_p, "bo": bo_eff,
                "ident": np.eye(128, dtype=WNP),
                "ones": np.ones((1, 16, 64), WNP),
                "cos": cos_t.astype(WNP), "sin": sin_t.astype(WNP),
            }
        )
    res = run_bass_kernel_spmd(nc, in_maps, list(range(NCORES)))
    out = np.concatenate([res.results[i]["out"] for i in range(NCORES)], axis=0)
    return out.reshape(B, NB, BS, F)


# revision 17
# speedup vs baseline: 1.0696x; 1.0146x over previous
"""Trainium2 Bass kernel for nn_MultiHeadDotProductAttention_14980845928960.

Block-local multi-head attention with partial RoPE:
  q/k/v projections -> RoPE on first 32 of 64 head dims -> softmax(QK^T/8)V
  -> output projection.  Shapes: inputs [4,16,256,1024], 16 heads x 64 dim,
  blocks of 256 tokens attend locally.

Strategy: data-parallel over the 64 (batch, block) pairs -> 8 blocks/core.
Projections are batched over PAIRS of blocks (512 tokens -> N=512 moving
operands); attention runs per 256-token block but with both blocks of a
pair packed into [128, 512] tiles.  The emission order SOFTWARE-PIPELINES
pairs: pair p's ScalarE-heavy attention chunks are interleaved between
pair p+1's PE-heavy projection phases, so the tensor engine never idles
long enough for the HAM clock gate to re-throttle it.
Everything keeps the contraction dim on SBUF partitions:
  - inputs DMA'd with fp32->bf16 cast (gpsimd SWDGE); x^T via REGULAR bf16
    matmuls against a bf16 identity (lhsT = x chunk) -- all-bf16 PE keeps
    fast-weight-load enabled; projections as lhsT=W chunk, rhs=x^T chunk.
  - Q/K channel-PERMUTED (host side) so rope dims occupy out-chunks 0-3
    and pass dims chunks 4-7; RoPE = R-matmul (pair swap w/ signs) + two
    elementwise multiplies with cos/sin tables (host-precomputed inputs).
  - scores computed TRANSPOSED (k on partitions) so no P transpose is
    needed; both blocks of a pair share one PSUM bank (cols 0:256 /
    256:512) so exp / 1/Z / normalize run as single [*, 512] ops; softmax
    needs no max-subtraction (scores ~N(0,1)); softmax denominators arrive
    replicated on PV-output partitions 64:128 via v_aug = [v_h | 1 x64];
    1/Z computed as exp(-ln Z) on ScalarE -- Ln and Exp share one
    activation-table set so no ACT_TABLE_LOAD churn; normalization folds
    into the attn PSUM->SBUF evacuation.
  - compute dtype bf16 (weights/x^T/q/k/P/v/attn), fp32 PSUM accumulate.
All scaling (1/sqrt(D)) and biases fold into host-prepped weights
(bq,bk folded on evac; bv,bo folded as bo_eff = bo + bv @ Wo since
softmax rows sum to one).
"""

import ml_dtypes
import numpy as np

import concourse.bass as bass
import concourse.tile as tile
from concourse import mybir
from concourse.bass_utils import run_bass_kernel_spmd

# ---------------------------------------------------------------- constants
B, NB, BS, F = 4, 16, 256, 1024
H, D, ROPE = 16, 64, 32
NCORES = 8
BLKS = B * NB                 # 64 blocks total
BPC = BLKS // NCORES          # 8 blocks per core
NPAIR = BPC // 2              # block pairs per core
BT = 2 * BS                   # tokens per pair (512)
F32 = mybir.dt.float32
BF16 = mybir.dt.bfloat16
WDT = BF16
WNP = ml_dtypes.bfloat16
MULT = mybir.AluOpType.mult
ADD = mybir.AluOpType.add
EXP = mybir.ActivationFunctionType.Exp
LN = mybir.ActivationFunctionType.Ln
IDENT_FN = mybir.ActivationFunctionType.Identity

# ------------------------------------------------- walrus multi-wait splitter
# This walrus build rejects >1 sync-wait per instruction on several
# instruction structs. Tile attaches several waits to one instruction;
# hoist extras onto NOPs inserted just before it on the same engine.
_split_ctr = [0]


def _split_multi_waits(nc, maxw=1):
    for f in nc.m.functions:
        for bb in f.blocks:
            insts = list(bb.instructions)
            out = []
            changed = False
            for inst in insts:
                si = inst.sync_info
                waits = list(si.on_wait) if si and si.on_wait else []
                if len(waits) > maxw:
                    changed = True
                    for w in waits[:-maxw]:
                        _split_ctr[0] += 1
                        nop = mybir.InstNoOp(
                            name=f"wsplit-{_split_ctr[0]}",
                            ins=[],
                            outs=[],
                            engine=inst.engine,
                        )
                        nop.sync_info = mybir.SyncInfo(on_wait=[w], on_update=[])
                        nc.register_instruction(nop)
                        out.append(nop)
                    si.on_wait = waits[-maxw:]
                out.append(inst)
            if changed:
                bb.instructions = out


# ---------------------------------------------------------------- bass build
class PairEmitter:
    """Emits one pair's work as phase closures so the caller can interleave
    pair p's attention chunks between pair p+1's projection phases."""

    def __init__(self, nc, pools, consts, pair, dram):
        self.nc = nc
        (self.psum, self.xin, self.xt, self.qk, self.ptp, self.attnp,
         self.outp, self.tabp) = pools
        (self.wq_sb, self.wk_sb, self.wv_sb, self.wo_sb, self.rt_sb,
         self.ident, self.bq_sb, self.bk_sb, self.bo_sb, self.vaug) = consts
        self.pair = pair
        (self.xq_d, self.xkv_d, self.cos_d, self.sin_d, self.out_d) = dram
        self.vaug_p = self.vaug[pair % 2]
        self.st = {}

    # ---- projection phases -------------------------------------------------
    def _transpose_input(self, x_d):
        nc = self.nc
        xt_in = {}
        for t in range(4):          # 4 token-chunks of 128
            for fh in range(2):
                xtile = self.xin.tile([128, 512], WDT, tag="xin",
                                      name=f"xin{t}{fh}")
                nc.gpsimd.dma_start(
                    out=xtile,
                    in_=x_d[
                        2 * self.pair + t // 2,
                        (t % 2) * 128 : (t % 2 + 1) * 128,
                        fh * 512 : (fh + 1) * 512,
                    ],
                )
                xt_in[(t, fh)] = xtile
        tiles = []
        for c in range(8):
            ps = self.psum.tile([128, BT], F32, tag="ps", bufs=5)
            for t in range(4):
                srct = xt_in[(t, c // 4)][:, (c % 4) * 128 : (c % 4 + 1) * 128]
                nc.tensor.matmul(
                    ps[:, t * 128 : (t + 1) * 128],
                    lhsT=srct,
                    rhs=self.ident,
                    start=True,
                    stop=True,
                )
            tt = self.xt.tile([128, BT], WDT, tag=f"xt{c}")
            nc.vector.tensor_copy(out=tt, in_=ps)
            tiles.append(tt)
        return tiles

    def _qk_proj(self, w_sb, b_sb, x_tiles, tagpfx):
        nc = self.nc
        cos_sb, sin_sb = self.st["cos"], self.st["sin"]
        outs = []
        for oc in range(8):
            ps = self.psum.tile([128, BT], F32, tag="ps", bufs=5)
            for c in range(8):
                nc.tensor.matmul(
                    ps,
                    lhsT=w_sb[c][:, oc * 128 : (oc + 1) * 128],
                    rhs=x_tiles[c],
                    start=(c == 0),
                    stop=(c == 7),
                )
            qf = self.qk.tile([128, BT], WDT, tag=f"{tagpfx}{oc}")
            if oc < 4:
                raw = self.qk.tile([128, BT], WDT, tag="raw", bufs=2)
                nc.vector.tensor_scalar_add(raw, ps, b_sb[:, oc : oc + 1])
                ps2 = self.psum.tile([128, BT], F32, tag="ps", bufs=5)
                nc.tensor.matmul(ps2, lhsT=self.rt_sb, rhs=raw,
                                 start=True, stop=True)
                qs2 = self.qk.tile([128, BT], WDT, tag="qs2", bufs=2)
                nc.vector.tensor_tensor(out=qs2, in0=ps2, in1=sin_sb, op=MULT)
                nc.gpsimd.tensor_tensor(out=qf, in0=raw, in1=cos_sb, op=MULT)
                nc.gpsimd.tensor_tensor(out=qf, in0=qf, in1=qs2, op=ADD)
            else:
                nc.vector.tensor_scalar_add(qf, ps, b_sb[:, oc : oc + 1])
            outs.append(qf)
        return outs

    def ph_tables_xq(self):
        nc = self.nc
        # gpsimd (SWDGE) queue: sync's queue head-of-line blocks behind the
        # previous pair's output stores, and rope needs these tables early.
        cos_sb = self.tabp.tile([128, BT], WDT, tag="cos", bufs=2)
        nc.gpsimd.dma_start(out=cos_sb, in_=self.cos_d[self.pair])
        sin_sb = self.tabp.tile([128, BT], WDT, tag="sin", bufs=2)
        nc.gpsimd.dma_start(out=sin_sb, in_=self.sin_d[self.pair])
        self.st["cos"], self.st["sin"] = cos_sb, sin_sb
        self.st["xqT"] = self._transpose_input(self.xq_d)

    def ph_qproj(self):
        self.st["qT"] = self._qk_proj(self.wq_sb, self.bq_sb,
                                      self.st["xqT"], "q")

    def ph_xkv(self):
        self.st["xkT"] = self._transpose_input(self.xkv_d)

    def ph_kproj(self):
        self.st["kT"] = self._qk_proj(self.wk_sb, self.bk_sb,
                                      self.st["xkT"], "k")

    def ph_vproj(self):
        # V projection into interleaved v_aug = [v_h | 1 x64] (128 cols/head).
        # The 64 ones-columns replicate the softmax row-sum onto PV output
        # partitions 64..127, already partition-broadcast for normalization.
        nc = self.nc
        xkT = self.st["xkT"]
        for kc in range(4):
            va = self.vaug_p[kc]
            va3 = va.rearrange("p (h c) -> p h c", c=128)
            for b2 in range(2):
                ps = self.psum.tile([128, 512], F32, tag="ps", bufs=5)
                for c in range(8):
                    nc.tensor.matmul(
                        ps,
                        lhsT=xkT[c][:, kc * 128 : (kc + 1) * 128],
                        rhs=self.wv_sb[c][:, b2 * 512 : (b2 + 1) * 512],
                        start=(c == 0),
                        stop=(c == 7),
                    )
                nc.vector.tensor_copy(
                    out=va3[:, b2 * 8 : (b2 + 1) * 8, 0:64],
                    in_=ps.rearrange("p (h c) -> p h c", c=64),
                )
        self.st["attnT"] = [
            self.attnp.tile([128, BT], WDT, tag=f"attnT{cc}",
                            name=f"attnT{cc}", bufs=2)
            for cc in range(8)
        ]

    def proj_phases(self):
        return [self.ph_tables_xq, self.ph_qproj, self.ph_xkv,
                self.ph_kproj, self.ph_vproj]

    # ---- attention chunks (per head-group) + output projection -------------
    def attn_hg(self, hg):
        """scores+exp+PV+normalize for heads 4hg..4hg+3, both blocks packed
        into [128, 512] tiles (cols 0:256 = block 0, 256:512 = block 1)."""
        nc = self.nc
        qT, kT = self.st["qT"], self.st["kT"]
        attnT = self.st["attnT"]
        rc, pc = hg, 4 + hg
        pts = {}
        for kc in range(2):
            sps = []
            for g in range(4):
                ps = self.psum.tile([128, BT], F32, tag="ps_att", bufs=3)
                r0 = 32 * g
                for qh in range(2):
                    ksl = slice((qh * 2 + kc) * 128, (qh * 2 + kc + 1) * 128)
                    qsl = slice(qh * 256, (qh + 1) * 256)
                    nc.tensor.matmul(
                        ps[:, qsl],
                        lhsT=kT[rc][r0 : r0 + 32, ksl],
                        rhs=qT[rc][r0 : r0 + 32, qsl],
                        start=True,
                        stop=False,
                        tile_position=(r0, 0),
                    )
                    nc.tensor.matmul(
                        ps[:, qsl],
                        lhsT=kT[pc][r0 : r0 + 32, ksl],
                        rhs=qT[pc][r0 : r0 + 32, qsl],
                        start=False,
                        stop=True,
                        tile_position=(r0, 0),
                    )
                sps.append(ps)
            for g in range(4):
                pt = self.ptp.tile(
                    [128, BT], WDT, tag=f"pt{g}_{kc}", name=f"pt{g}_{kc}",
                    bufs=2,
                )
                nc.scalar.activation(out=pt, in_=sps[g], func=EXP)
                pts[(g, kc)] = pt
        # PV + 1/Z (= exp(-ln Z), same act-table set) + normalized evacuation
        for g in range(4):
            h = 4 * hg + g
            aps = self.psum.tile([128, BT], F32, tag="ps_att", bufs=3)
            for qh in range(2):
                qsl = slice(qh * 256, (qh + 1) * 256)
                for kc in range(2):
                    nc.tensor.matmul(
                        aps[:, qsl],
                        lhsT=self.vaug_p[qh * 2 + kc][:, h * 128 : (h + 1) * 128],
                        rhs=pts[(g, kc)][:, qsl],
                        start=(kc == 0),
                        stop=(kc == 1),
                    )
            lnz = self.attnp.tile([64, BT], F32, tag="lnz", bufs=2)
            nc.scalar.activation(out=lnz, in_=aps[64:128, :], func=LN)
            rec = self.attnp.tile([64, BT], F32, tag="recip", bufs=2)
            nc.scalar.activation(out=rec, in_=lnz, func=EXP, scale=-1.0)
            cc, r0 = h // 2, (h % 2) * 64
            nc.vector.tensor_tensor(
                out=attnT[cc][r0 : r0 + 64, :],
                in0=aps[0:64, :],
                in1=rec,
                op=MULT,
            )

    def out_proj(self):
        nc = self.nc
        attnT = self.st["attnT"]
        for t2 in range(4):
            for n2 in range(2):
                ps = self.psum.tile([128, 512], F32, tag="ps", bufs=5)
                for cc in range(8):
                    nc.tensor.matmul(
                        ps,
                        lhsT=attnT[cc][:, t2 * 128 : (t2 + 1) * 128],
                        rhs=self.wo_sb[cc][:, n2 * 512 : (n2 + 1) * 512],
                        start=(cc == 0),
                        stop=(cc == 7),
                    )
                ob = self.outp.tile([128, 512], F32, tag="outsb")
                nc.vector.tensor_tensor(
                    out=ob,
                    in0=ps,
                    in1=self.bo_sb[:, n2 * 512 : (n2 + 1) * 512],
                    op=ADD,
                )
                nc.sync.dma_start(
                    out=self.out_d[
                        2 * self.pair + t2 // 2,
                        (t2 % 2) * 128 : (t2 % 2 + 1) * 128,
                        n2 * 512 : (n2 + 1) * 512,
                    ],
                    in_=ob,
                )

    def tail_chunks(self):
        return [lambda hg=hg: self.attn_hg(hg) for hg in range(4)] + [
            self.out_proj
        ]


def build_program():
    nc = bass.Bass("TRN2")
    xq_d = nc.dram_tensor("xq", [BPC, BS, F], F32, kind="ExternalInput")
    xkv_d = nc.dram_tensor("xkv", [BPC, BS, F], F32, kind="ExternalInput")
    wq_d = nc.dram_tensor("wq", [8, 128, F], WDT, kind="ExternalInput")
    wk_d = nc.dram_tensor("wk", [8, 128, F], WDT, kind="ExternalInput")
    wv_d = nc.dram_tensor("wv", [8, 128, F], WDT, kind="ExternalInput")
    wo_d = nc.dram_tensor("wo", [8, 128, F], WDT, kind="ExternalInput")
    rt_d = nc.dram_tensor("rt", [128, 128], WDT, kind="ExternalInput")
    ident_d = nc.dram_tensor("ident", [128, 128], WDT, kind="ExternalInput")
    ones_d = nc.dram_tensor("ones", [1, 16, 64], WDT, kind="ExternalInput")
    bq_d = nc.dram_tensor("bq", [128, 8], F32, kind="ExternalInput")
    bk_d = nc.dram_tensor("bk", [128, 8], F32, kind="ExternalInput")
    bo_d = nc.dram_tensor("bo", [1, F], F32, kind="ExternalInput")
    cos_d = nc.dram_tensor("cos", [NPAIR, 128, BT], WDT, kind="ExternalInput")
    sin_d = nc.dram_tensor("sin", [NPAIR, 128, BT], WDT, kind="ExternalInput")
    out_d = nc.dram_tensor("out", [BPC, BS, F], F32, kind="ExternalOutput")

    with tile.TileContext(nc) as tc:
        with (
            tc.tile_pool(name="wpool", bufs=1) as wpool,
            tc.tile_pool(name="psum", bufs=8, space="PSUM") as psum,
            tc.tile_pool(name="xin", bufs=4) as xin,
            tc.tile_pool(name="xt", bufs=2) as xt,
            tc.tile_pool(name="qk", bufs=2) as qk,
            tc.tile_pool(name="ptp", bufs=1) as ptp,
            tc.tile_pool(name="attnp", bufs=1) as attnp,
            tc.tile_pool(name="outp", bufs=2) as outp,
            tc.tile_pool(name="tabp", bufs=1) as tabp,
        ):
            def wtiles(src, tagpfx):
                ts = []
                for c in range(8):
                    t = wpool.tile([128, F], WDT, tag=f"{tagpfx}{c}", name=f"{tagpfx}{c}")
                    nc.sync.dma_start(out=t, in_=src[c])
                    ts.append(t)
                return ts

            # load order matters on the sync queue: first pair needs ident
            # (transposes), wq, rope consts before anything else.
            ident = wpool.tile([128, 128], WDT, tag="ident")
            nc.sync.dma_start(out=ident, in_=ident_d[:])
            rt_sb = wpool.tile([128, 128], WDT, tag="rt")
            nc.sync.dma_start(out=rt_sb, in_=rt_d[:])
            bq_sb = wpool.tile([128, 8], F32, tag="bq")
            nc.sync.dma_start(out=bq_sb, in_=bq_d[:])
            bk_sb = wpool.tile([128, 8], F32, tag="bk")
            nc.sync.dma_start(out=bk_sb, in_=bk_d[:])
            wq_sb = wtiles(wq_d, "wq")
            wk_sb = wtiles(wk_d, "wk")
            wv_sb = wtiles(wv_d, "wv")
            wo_sb = wtiles(wo_d, "wo")
            bo_sb = wpool.tile([128, F], F32, tag="bo")
            nc.sync.dma_start(out=bo_sb, in_=bo_d[0:1, :].to_broadcast([128, F]))

            vaug = []
            for par in range(2):
                vset = []
                for kc in range(4):
                    va = wpool.tile(
                        [128, 2048], WDT,
                        tag=f"vaug{par}{kc}", name=f"vaug{par}{kc}",
                    )
                    nc.sync.dma_start(
                        out=va.rearrange("p (h c) -> p h c", c=128)[:, :, 64:128],
                        in_=ones_d[:].to_broadcast([128, 16, 64]),
                    )
                    vset.append(va)
                vaug.append(vset)

            pools = (psum, xin, xt, qk, ptp, attnp, outp, tabp)
            consts = (
                wq_sb, wk_sb, wv_sb, wo_sb, rt_sb, ident, bq_sb, bk_sb, bo_sb, vaug
            )
            dram = (xq_d, xkv_d, cos_d, sin_d, out_d)

            # software pipeline: pair p's attention/output chunks emitted
            # between pair p+1's projection phases.
            prev_tail = []
            for pair in range(NPAIR):
                em = PairEmitter(nc, pools, consts, pair, dram)
                phases = em.proj_phases()
                for i, ph in enumerate(phases):
                    ph()
                    if i < len(prev_tail):
                        prev_tail[i]()
                for c in prev_tail[len(phases):]:
                    c()
                prev_tail = em.tail_chunks()
            for c in prev_tail:
                c()

    _split_multi_waits(nc)
    return nc


# ---------------------------------------------------------------- host side
def _host_prep(Wq, bq, Wk, bk, Wv, bv, Wo, bo):
    """Permute/scale weights; fold biases."""
    old_of_new = np.empty(F, np.int64)
    for h in range(H):
        old_of_new[h * ROPE : (h + 1) * ROPE] = h * D + np.arange(ROPE)
        old_of_new[512 + h * ROPE : 512 + (h + 1) * ROPE] = (
            h * D + ROPE + np.arange(ROPE)
        )
    wq_flat = (Wq.reshape(F, F) / np.sqrt(D)).astype(np.float32)
    wq_p = np.ascontiguousarray(wq_flat[:, old_of_new]).reshape(8, 128, F)
    wk_flat = Wk.reshape(F, F).astype(np.float32)
    wk_p = np.ascontiguousarray(wk_flat[:, old_of_new]).reshape(8, 128, F)
    wv_c = np.ascontiguousarray(Wv.reshape(F, F)).reshape(8, 128, F)
    wo_c = np.ascontiguousarray(Wo.reshape(F, F)).reshape(8, 128, F)
    bq_p = np.ascontiguousarray(
        (bq.reshape(F) / np.sqrt(D))[old_of_new].reshape(8, 128).T
    ).astype(np.float32)
    bk_p = np.ascontiguousarray(bk.reshape(F)[old_of_new].reshape(8, 128).T).astype(
        np.float32
    )
    bo_eff = (bo + bv.reshape(F) @ Wo.reshape(F, F)).reshape(1, F).astype(np.float32)

    # R^T for rotate_every_two with signs: (R@q)[2i] = -q[2i+1]; [2i+1] = q[2i]
    R = np.zeros((128, 128), np.float32)
    for g in range(4):          # 4 heads per rope chunk, 32 rows each
        for i in range(ROPE // 2):
            R[g * 32 + 2 * i, g * 32 + 2 * i + 1] = -1.0
            R[g * 32 + 2 * i + 1, g * 32 + 2 * i] = 1.0
    rt = np.ascontiguousarray(R.T)
    return wq_p, wk_p, wv_c, wo_c, bq_p, bk_p, bo_eff, rt


def _tables_for_core(core):
    """cos/sin tables [NPAIR, 128, 512] for this core's block pairs."""
    inv_freq = 1.0 / 10000.0 ** (np.arange(0, ROPE, 2) / ROPE)
    cos_t = np.empty((NPAIR, 128, BT), np.float32)
    sin_t = np.empty((NPAIR, 128, BT), np.float32)
    for p in range(NPAIR):
        for half in range(2):
            nb = (core * BPC + 2 * p + half) % NB
            pos = nb * BS + np.arange(BS, dtype=np.float64)
            ang = pos[None, :] * inv_freq[:, None]          # [16, 256]
            cpat = np.repeat(np.cos(ang), 2, axis=0)        # [32, 256]
            spat = np.repeat(np.sin(ang), 2, axis=0)
            sl = slice(half * BS, (half + 1) * BS)
            cos_t[p, :, sl] = np.tile(cpat, (4, 1))
            sin_t[p, :, sl] = np.tile(spat, (4, 1))
    return cos_t, sin_t


_nc_cache = []


def kernel(inputs_q, inputs_kv, Wq, bq, Wk, bk, Wv, bv, Wo, bo):
    inputs_q = np.asarray(inputs_q, np.float32)
    inputs_kv = np.asarray(inputs_kv, np.float32)
    wq_p, wk_p, wv_c, wo_c, bq_p, bk_p, bo_eff, rt = _host_prep(
        np.asarray(Wq), np.asarray(bq), np.asarray(Wk), np.asarray(bk),
        np.asarray(Wv), np.asarray(bv), np.asarray(Wo), np.asarray(bo),
    )
    xq_all = inputs_q.reshape(BLKS, BS, F)
    xkv_all = inputs_kv.reshape(BLKS, BS, F)
    wq_p = wq_p.astype(WNP)
    wk_p = wk_p.astype(WNP)
    wv_c = wv_c.astype(WNP)
    wo_c = wo_c.astype(WNP)

    if not _nc_cache:
        _nc_cache.append(build_program())
    nc = _nc_cache[0]

    in_maps = []
    for core in range(NCORES):
        cos_t, sin_t = _tables_for_core(core)
        in_maps.append(
            {
                "xq": np.ascontiguousarray(xq_all[core * BPC : (core + 1) * BPC]),
                "xkv": np.ascontiguousarray(xkv_all[core * BPC : (core + 1) * BPC]),
                "wq": wq_p, "wk": wk_p, "wv": wv_c, "wo": wo_c,
                "rt": rt.astype(WNP), "bq": bq_p, "bk": bk_p, "bo": bo_eff,
                "ident": np.eye(128, dtype=WNP),
                "ones": np.ones((1, 16, 64), WNP),
                "cos": cos_t.astype(WNP), "sin": sin_t.astype(WNP),
            }
        )
    res = run_bass_kernel_spmd(nc, in_maps, list(range(NCORES)))
    out = np.concatenate([res.results[i]["out"] for i in range(NCORES)], axis=0)
    return out.reshape(B, NB, BS, F)


# revision 18
# speedup vs baseline: 1.0821x; 1.0117x over previous
"""Trainium2 Bass kernel for nn_MultiHeadDotProductAttention_14980845928960.

Block-local multi-head attention with partial RoPE:
  q/k/v projections -> RoPE on first 32 of 64 head dims -> softmax(QK^T/8)V
  -> output projection.  Shapes: inputs [4,16,256,1024], 16 heads x 64 dim,
  blocks of 256 tokens attend locally.

Strategy: data-parallel over the 64 (batch, block) pairs -> 8 blocks/core.
Projections are batched over PAIRS of blocks (512 tokens -> N=512 moving
operands); attention runs per 256-token block but with both blocks of a
pair packed into [128, 512] tiles.  The emission order SOFTWARE-PIPELINES
pairs: pair p's ScalarE-heavy attention chunks are interleaved between
pair p+1's PE-heavy projection phases, so the tensor engine never idles
long enough for the HAM clock gate to re-throttle it.
Everything keeps the contraction dim on SBUF partitions:
  - inputs DMA'd with fp32->bf16 cast (gpsimd SWDGE); x^T via REGULAR bf16
    matmuls against a bf16 identity (lhsT = x chunk) -- all-bf16 PE keeps
    fast-weight-load enabled; projections as lhsT=W chunk, rhs=x^T chunk.
  - Q/K channel-PERMUTED (host side) so rope dims occupy out-chunks 0-3
    and pass dims chunks 4-7; RoPE = R-matmul (pair swap w/ signs) + two
    elementwise multiplies with cos/sin tables (host-precomputed inputs).
  - scores computed TRANSPOSED (k on partitions) so no P transpose is
    needed; both blocks of a pair share one PSUM bank (cols 0:256 /
    256:512) so exp / 1/Z / normalize run as single [*, 512] ops; softmax
    needs no max-subtraction (scores ~N(0,1)); softmax denominators arrive
    replicated on PV-output partitions 64:128 via v_aug = [v_h | 1 x64];
    1/Z computed as exp(-ln Z) on ScalarE -- Ln and Exp share one
    activation-table set so no ACT_TABLE_LOAD churn; normalization folds
    into the attn PSUM->SBUF evacuation.
  - compute dtype bf16 (weights/x^T/q/k/P/v/attn), fp32 PSUM accumulate.
All scaling (1/sqrt(D)) and biases fold into host-prepped weights
(bq,bk folded on evac; bv,bo folded as bo_eff = bo + bv @ Wo since
softmax rows sum to one).
"""

import ml_dtypes
import numpy as np

import concourse.bass as bass
import concourse.tile as tile
from concourse import mybir
from concourse.bass_utils import run_bass_kernel_spmd

# ---------------------------------------------------------------- constants
B, NB, BS, F = 4, 16, 256, 1024
H, D, ROPE = 16, 64, 32
NCORES = 8
BLKS = B * NB                 # 64 blocks total
BPC = BLKS // NCORES          # 8 blocks per core
NPAIR = BPC // 2              # block pairs per core
BT = 2 * BS                   # tokens per pair (512)
F32 = mybir.dt.float32
BF16 = mybir.dt.bfloat16
WDT = BF16
WNP = ml_dtypes.bfloat16
MULT = mybir.AluOpType.mult
ADD = mybir.AluOpType.add
EXP = mybir.ActivationFunctionType.Exp
LN = mybir.ActivationFunctionType.Ln
IDENT_FN = mybir.ActivationFunctionType.Identity

# ------------------------------------------------- walrus multi-wait splitter
# This walrus build rejects >1 sync-wait per instruction on several
# instruction structs. Tile attaches several waits to one instruction;
# hoist extras onto NOPs inserted just before it on the same engine.
_split_ctr = [0]


def _split_multi_waits(nc, maxw=1):
    for f in nc.m.functions:
        for bb in f.blocks:
            insts = list(bb.instructions)
            out = []
            changed = False
            for inst in insts:
                si = inst.sync_info
                waits = list(si.on_wait) if si and si.on_wait else []
                if len(waits) > maxw:
                    changed = True
                    for w in waits[:-maxw]:
                        _split_ctr[0] += 1
                        nop = mybir.InstNoOp(
                            name=f"wsplit-{_split_ctr[0]}",
                            ins=[],
                            outs=[],
                            engine=inst.engine,
                        )
                        nop.sync_info = mybir.SyncInfo(on_wait=[w], on_update=[])
                        nc.register_instruction(nop)
                        out.append(nop)
                    si.on_wait = waits[-maxw:]
                out.append(inst)
            if changed:
                bb.instructions = out


# ---------------------------------------------------------------- bass build
class PairEmitter:
    """Emits one pair's work as phase closures so the caller can interleave
    pair p's attention chunks between pair p+1's projection phases."""

    def __init__(self, nc, pools, consts, pair, dram):
        self.nc = nc
        (self.psum, self.xin, self.xt, self.qk, self.ptp, self.attnp,
         self.outp, self.tabp) = pools
        (self.wq_sb, self.wk_sb, self.wv_sb, self.wo_sb, self.rt_sb,
         self.ident, self.bq_sb, self.bk_sb, self.bo_sb, self.vaug) = consts
        self.pair = pair
        (self.xq_d, self.xkv_d, self.cos_d, self.sin_d, self.out_d) = dram
        self.vaug_p = self.vaug[pair % 2]
        self.st = {}

    # ---- projection phases -------------------------------------------------
    def _transpose_input(self, x_d):
        nc = self.nc
        xt_in = {}
        for t in range(4):          # 4 token-chunks of 128
            for fh in range(2):
                xtile = self.xin.tile([128, 512], WDT, tag="xin",
                                      name=f"xin{t}{fh}")
                nc.gpsimd.dma_start(
                    out=xtile,
                    in_=x_d[
                        2 * self.pair + t // 2,
                        (t % 2) * 128 : (t % 2 + 1) * 128,
                        fh * 512 : (fh + 1) * 512,
                    ],
                )
                xt_in[(t, fh)] = xtile
        tiles = []
        for c in range(8):
            ps = self.psum.tile([128, BT], F32, tag="ps", bufs=4)
            for t in range(4):
                srct = xt_in[(t, c // 4)][:, (c % 4) * 128 : (c % 4 + 1) * 128]
                nc.tensor.matmul(
                    ps[:, t * 128 : (t + 1) * 128],
                    lhsT=srct,
                    rhs=self.ident,
                    start=True,
                    stop=True,
                )
            tt = self.xt.tile([128, BT], WDT, tag=f"xt{c}")
            nc.vector.tensor_copy(out=tt, in_=ps)
            tiles.append(tt)
        return tiles

    def _qk_proj(self, w_sb, b_sb, x_tiles, tagpfx):
        nc = self.nc
        cos_sb, sin_sb = self.st["cos"], self.st["sin"]
        outs = []
        for oc in range(8):
            ps = self.psum.tile([128, BT], F32, tag="ps", bufs=4)
            for c in range(8):
                nc.tensor.matmul(
                    ps,
                    lhsT=w_sb[c][:, oc * 128 : (oc + 1) * 128],
                    rhs=x_tiles[c],
                    start=(c == 0),
                    stop=(c == 7),
                )
            qf = self.qk.tile([128, BT], WDT, tag=f"{tagpfx}{oc}")
            if oc < 4:
                raw = self.qk.tile([128, BT], WDT, tag="raw", bufs=2)
                nc.vector.tensor_scalar_add(raw, ps, b_sb[:, oc : oc + 1])
                ps2 = self.psum.tile([128, BT], F32, tag="ps", bufs=4)
                nc.tensor.matmul(ps2, lhsT=self.rt_sb, rhs=raw,
                                 start=True, stop=True)
                qs2 = self.qk.tile([128, BT], WDT, tag="qs2", bufs=2)
                nc.vector.tensor_tensor(out=qs2, in0=ps2, in1=sin_sb, op=MULT)
                nc.gpsimd.tensor_tensor(out=qf, in0=raw, in1=cos_sb, op=MULT)
                nc.gpsimd.tensor_tensor(out=qf, in0=qf, in1=qs2, op=ADD)
            else:
                nc.vector.tensor_scalar_add(qf, ps, b_sb[:, oc : oc + 1])
            outs.append(qf)
        return outs

    def ph_tables_xq(self):
        nc = self.nc
        # gpsimd (SWDGE) queue: sync's queue head-of-line blocks behind the
        # previous pair's output stores, and rope needs these tables early.
        cos_sb = self.tabp.tile([128, BT], WDT, tag="cos", bufs=2)
        nc.gpsimd.dma_start(out=cos_sb, in_=self.cos_d[self.pair])
        sin_sb = self.tabp.tile([128, BT], WDT, tag="sin", bufs=2)
        nc.gpsimd.dma_start(out=sin_sb, in_=self.sin_d[self.pair])
        self.st["cos"], self.st["sin"] = cos_sb, sin_sb
        self.st["xqT"] = self._transpose_input(self.xq_d)

    def ph_qproj(self):
        self.st["qT"] = self._qk_proj(self.wq_sb, self.bq_sb,
                                      self.st["xqT"], "q")

    def ph_xkv(self):
        self.st["xkT"] = self._transpose_input(self.xkv_d)

    def ph_kproj(self):
        self.st["kT"] = self._qk_proj(self.wk_sb, self.bk_sb,
                                      self.st["xkT"], "k")

    def ph_vproj(self):
        # V projection into interleaved v_aug = [v_h | 1 x64] (128 cols/head).
        # The 64 ones-columns replicate the softmax row-sum onto PV output
        # partitions 64..127, already partition-broadcast for normalization.
        nc = self.nc
        xkT = self.st["xkT"]
        for kc in range(4):
            va = self.vaug_p[kc]
            va3 = va.rearrange("p (h c) -> p h c", c=128)
            for b2 in range(2):
                ps = self.psum.tile([128, 512], F32, tag="ps", bufs=4)
                for c in range(8):
                    nc.tensor.matmul(
                        ps,
                        lhsT=xkT[c][:, kc * 128 : (kc + 1) * 128],
                        rhs=self.wv_sb[c][:, b2 * 512 : (b2 + 1) * 512],
                        start=(c == 0),
                        stop=(c == 7),
                    )
                nc.vector.tensor_copy(
                    out=va3[:, b2 * 8 : (b2 + 1) * 8, 0:64],
                    in_=ps.rearrange("p (h c) -> p h c", c=64),
                )
        self.st["attnT"] = [
            self.attnp.tile([128, BT], WDT, tag=f"attnT{cc}",
                            name=f"attnT{cc}", bufs=2)
            for cc in range(8)
        ]

    def proj_phases(self):
        return [self.ph_tables_xq, self.ph_qproj, self.ph_xkv,
                self.ph_kproj, self.ph_vproj]

    # ---- attention chunks (per head-group) + output projection -------------
    def attn_hg(self, hg):
        """scores+exp+PV+normalize for heads 4hg..4hg+3; PSUM tiles span TWO
        banks ([128,1024] f32, one matmul per bank-half) so each ScalarE
        activation covers two heads."""
        nc = self.nc
        qT, kT = self.st["qT"], self.st["kT"]
        attnT = self.st["attnT"]
        rc, pc = hg, 4 + hg
        pts = {}
        for kc in range(2):
            for gp in range(2):     # head-pairs (g = 2gp, 2gp+1)
                ps = self.psum.tile([128, 2 * BT], F32, tag="ps_att", bufs=2)
                for gi in range(2):
                    g = 2 * gp + gi
                    r0 = 32 * g
                    for qh in range(2):
                        ksl = slice((qh * 2 + kc) * 128, (qh * 2 + kc + 1) * 128)
                        qsl = slice(gi * BT + qh * 256, gi * BT + (qh + 1) * 256)
                        qsl_q = slice(qh * 256, (qh + 1) * 256)
                        nc.tensor.matmul(
                            ps[:, qsl],
                            lhsT=kT[rc][r0 : r0 + 32, ksl],
                            rhs=qT[rc][r0 : r0 + 32, qsl_q],
                            start=True,
                            stop=False,
                            tile_position=(r0, 0),
                        )
                        nc.tensor.matmul(
                            ps[:, qsl],
                            lhsT=kT[pc][r0 : r0 + 32, ksl],
                            rhs=qT[pc][r0 : r0 + 32, qsl_q],
                            start=False,
                            stop=True,
                            tile_position=(r0, 0),
                        )
                pt = self.ptp.tile(
                    [128, 2 * BT], WDT, tag=f"pt{gp}_{kc}", name=f"pt{gp}_{kc}",
                    bufs=2,
                )
                nc.scalar.activation(out=pt, in_=ps, func=EXP)
                for gi in range(2):
                    pts[(2 * gp + gi, kc)] = pt[:, gi * BT : (gi + 1) * BT]
        # PV (head-pair per two-bank tile) + 1/Z (= exp(-ln Z), same
        # act-table set, two heads per activation) + normalized evacuation
        for gp in range(2):
            aps = self.psum.tile([128, 2 * BT], F32, tag="ps_att", bufs=2)
            for gi in range(2):
                g = 2 * gp + gi
                h = 4 * hg + g
                for qh in range(2):
                    qsl = slice(gi * BT + qh * 256, gi * BT + (qh + 1) * 256)
                    qsl_q = slice(qh * 256, (qh + 1) * 256)
                    for kc in range(2):
                        nc.tensor.matmul(
                            aps[:, qsl],
                            lhsT=self.vaug_p[qh * 2 + kc][:, h * 128 : (h + 1) * 128],
                            rhs=pts[(g, kc)][:, qsl_q],
                            start=(kc == 0),
                            stop=(kc == 1),
                        )
            lnz = self.attnp.tile([64, 2 * BT], F32, tag="lnz", bufs=1)
            nc.scalar.activation(out=lnz, in_=aps[64:128, :], func=LN)
            rec = self.attnp.tile([64, 2 * BT], WDT, tag="recip", bufs=2)
            nc.scalar.activation(out=rec, in_=lnz, func=EXP, scale=-1.0)
            for gi in range(2):
                h = 4 * hg + 2 * gp + gi
                cc, r0 = h // 2, (h % 2) * 64
                nc.vector.tensor_tensor(
                    out=attnT[cc][r0 : r0 + 64, :],
                    in0=aps[0:64, gi * BT : (gi + 1) * BT],
                    in1=rec[:, gi * BT : (gi + 1) * BT],
                    op=MULT,
                )

    def out_proj(self):
        nc = self.nc
        attnT = self.st["attnT"]
        for t2 in range(4):
            for n2 in range(2):
                ps = self.psum.tile([128, 512], F32, tag="ps", bufs=4)
                for cc in range(8):
                    nc.tensor.matmul(
                        ps,
                        lhsT=attnT[cc][:, t2 * 128 : (t2 + 1) * 128],
                        rhs=self.wo_sb[cc][:, n2 * 512 : (n2 + 1) * 512],
                        start=(cc == 0),
                        stop=(cc == 7),
                    )
                ob = self.outp.tile([128, 512], F32, tag="outsb")
                nc.vector.tensor_tensor(
                    out=ob,
                    in0=ps,
                    in1=self.bo_sb[:, n2 * 512 : (n2 + 1) * 512],
                    op=ADD,
                )
                nc.sync.dma_start(
                    out=self.out_d[
                        2 * self.pair + t2 // 2,
                        (t2 % 2) * 128 : (t2 % 2 + 1) * 128,
                        n2 * 512 : (n2 + 1) * 512,
                    ],
                    in_=ob,
                )

    def tail_chunks(self):
        return [lambda hg=hg: self.attn_hg(hg) for hg in range(4)] + [
            self.out_proj
        ]


def build_program():
    nc = bass.Bass("TRN2")
    xq_d = nc.dram_tensor("xq", [BPC, BS, F], F32, kind="ExternalInput")
    xkv_d = nc.dram_tensor("xkv", [BPC, BS, F], F32, kind="ExternalInput")
    wq_d = nc.dram_tensor("wq", [8, 128, F], WDT, kind="ExternalInput")
    wk_d = nc.dram_tensor("wk", [8, 128, F], WDT, kind="ExternalInput")
    wv_d = nc.dram_tensor("wv", [8, 128, F], WDT, kind="ExternalInput")
    wo_d = nc.dram_tensor("wo", [8, 128, F], WDT, kind="ExternalInput")
    rt_d = nc.dram_tensor("rt", [128, 128], WDT, kind="ExternalInput")
    ident_d = nc.dram_tensor("ident", [128, 128], WDT, kind="ExternalInput")
    ones_d = nc.dram_tensor("ones", [1, 16, 64], WDT, kind="ExternalInput")
    bq_d = nc.dram_tensor("bq", [128, 8], F32, kind="ExternalInput")
    bk_d = nc.dram_tensor("bk", [128, 8], F32, kind="ExternalInput")
    bo_d = nc.dram_tensor("bo", [1, F], F32, kind="ExternalInput")
    cos_d = nc.dram_tensor("cos", [NPAIR, 128, BT], WDT, kind="ExternalInput")
    sin_d = nc.dram_tensor("sin", [NPAIR, 128, BT], WDT, kind="ExternalInput")
    out_d = nc.dram_tensor("out", [BPC, BS, F], F32, kind="ExternalOutput")

    with tile.TileContext(nc) as tc:
        with (
            tc.tile_pool(name="wpool", bufs=1) as wpool,
            tc.tile_pool(name="psum", bufs=8, space="PSUM") as psum,
            tc.tile_pool(name="xin", bufs=4) as xin,
            tc.tile_pool(name="xt", bufs=2) as xt,
            tc.tile_pool(name="qk", bufs=2) as qk,
            tc.tile_pool(name="ptp", bufs=1) as ptp,
            tc.tile_pool(name="attnp", bufs=1) as attnp,
            tc.tile_pool(name="outp", bufs=2) as outp,
            tc.tile_pool(name="tabp", bufs=1) as tabp,
        ):
            def wtiles(src, tagpfx):
                ts = []
                for c in range(8):
                    t = wpool.tile([128, F], WDT, tag=f"{tagpfx}{c}", name=f"{tagpfx}{c}")
                    nc.sync.dma_start(out=t, in_=src[c])
                    ts.append(t)
                return ts

            # load order matters on the sync queue: first pair needs ident
            # (transposes), wq, rope consts before anything else.
            ident = wpool.tile([128, 128], WDT, tag="ident")
            nc.sync.dma_start(out=ident, in_=ident_d[:])
            rt_sb = wpool.tile([128, 128], WDT, tag="rt")
            nc.sync.dma_start(out=rt_sb, in_=rt_d[:])
            bq_sb = wpool.tile([128, 8], F32, tag="bq")
            nc.sync.dma_start(out=bq_sb, in_=bq_d[:])
            bk_sb = wpool.tile([128, 8], F32, tag="bk")
            nc.sync.dma_start(out=bk_sb, in_=bk_d[:])
            wq_sb = wtiles(wq_d, "wq")
            wk_sb = wtiles(wk_d, "wk")
            wv_sb = wtiles(wv_d, "wv")
            wo_sb = wtiles(wo_d, "wo")
            bo_sb = wpool.tile([128, F], F32, tag="bo")
            nc.sync.dma_start(out=bo_sb, in_=bo_d[0:1, :].to_broadcast([128, F]))

            vaug = []
            for par in range(2):
                vset = []
                for kc in range(4):
                    va = wpool.tile(
                        [128, 2048], WDT,
                        tag=f"vaug{par}{kc}", name=f"vaug{par}{kc}",
                    )
                    nc.sync.dma_start(
                        out=va.rearrange("p (h c) -> p h c", c=128)[:, :, 64:128],
                        in_=ones_d[:].to_broadcast([128, 16, 64]),
                    )
                    vset.append(va)
                vaug.append(vset)

            pools = (psum, xin, xt, qk, ptp, attnp, outp, tabp)
            consts = (
                wq_sb, wk_sb, wv_sb, wo_sb, rt_sb, ident, bq_sb, bk_sb, bo_sb, vaug
            )
            dram = (xq_d, xkv_d, cos_d, sin_d, out_d)

            # software pipeline: pair p's attention/output chunks emitted
            # between pair p+1's projection phases.
            prev_tail = []
            for pair in range(NPAIR):
                em = PairEmitter(nc, pools, consts, pair, dram)
                phases = em.proj_phases()
                for i, ph in enumerate(phases):
                    ph()
                    if i < len(prev_tail):
                        prev_tail[i]()
                for c in prev_tail[len(phases):]:
                    c()
                prev_tail = em.tail_chunks()
            for c in prev_tail:
                c()

    _split_multi_waits(nc)
    return nc


# ---------------------------------------------------------------- host side
def _host_prep(Wq, bq, Wk, bk, Wv, bv, Wo, bo):
    """Permute/scale weights; fold biases."""
    old_of_new = np.empty(F, np.int64)
    for h in range(H):
        old_of_new[h * ROPE : (h + 1) * ROPE] = h * D + np.arange(ROPE)
        old_of_new[512 + h * ROPE : 512 + (h + 1) * ROPE] = (
            h * D + ROPE + np.arange(ROPE)
        )
    wq_flat = (Wq.reshape(F, F) / np.sqrt(D)).astype(np.float32)
    wq_p = np.ascontiguousarray(wq_flat[:, old_of_new]).reshape(8, 128, F)
    wk_flat = Wk.reshape(F, F).astype(np.float32)
    wk_p = np.ascontiguousarray(wk_flat[:, old_of_new]).reshape(8, 128, F)
    wv_c = np.ascontiguousarray(Wv.reshape(F, F)).reshape(8, 128, F)
    wo_c = np.ascontiguousarray(Wo.reshape(F, F)).reshape(8, 128, F)
    bq_p = np.ascontiguousarray(
        (bq.reshape(F) / np.sqrt(D))[old_of_new].reshape(8, 128).T
    ).astype(np.float32)
    bk_p = np.ascontiguousarray(bk.reshape(F)[old_of_new].reshape(8, 128).T).astype(
        np.float32
    )
    bo_eff = (bo + bv.reshape(F) @ Wo.reshape(F, F)).reshape(1, F).astype(np.float32)

    # R^T for rotate_every_two with signs: (R@q)[2i] = -q[2i+1]; [2i+1] = q[2i]
    R = np.zeros((128, 128), np.float32)
    for g in range(4):          # 4 heads per rope chunk, 32 rows each
        for i in range(ROPE // 2):
            R[g * 32 + 2 * i, g * 32 + 2 * i + 1] = -1.0
            R[g * 32 + 2 * i + 1, g * 32 + 2 * i] = 1.0
    rt = np.ascontiguousarray(R.T)
    return wq_p, wk_p, wv_c, wo_c, bq_p, bk_p, bo_eff, rt


def _tables_for_core(core):
    """cos/sin tables [NPAIR, 128, 512] for this core's block pairs."""
    inv_freq = 1.0 / 10000.0 ** (np.arange(0, ROPE, 2) / ROPE)
    cos_t = np.empty((NPAIR, 128, BT), np.float32)
    sin_t = np.empty((NPAIR, 128, BT), np.float32)
    for p in range(NPAIR):
        for half in range(2):
            nb = (core * BPC + 2 * p + half) % NB
            pos = nb * BS + np.arange(BS, dtype=np.float64)
            ang = pos[None, :] * inv_freq[:, None]          # [16, 256]
            cpat = np.repeat(np.cos(ang), 2, axis=0)        # [32, 256]
            spat = np.repeat(np.sin(ang), 2, axis=0)
            sl = slice(half * BS, (half + 1) * BS)
            cos_t[p, :, sl] = np.tile(cpat, (4, 1))
            sin_t[p, :, sl] = np.tile(spat, (4, 1))
    return cos_t, sin_t


_nc_cache = []


def kernel(inputs_q, inputs_kv, Wq, bq, Wk, bk, Wv, bv, Wo, bo):
    inputs_q = np.asarray(inputs_q, np.float32)
    inputs_kv = np.asarray(inputs_kv, np.float32)
    wq_p, wk_p, wv_c, wo_c, bq_p, bk_p, bo_eff, rt = _host_prep(
        np.asarray(Wq), np.asarray(bq), np.asarray(Wk), np.asarray(bk),
        np.asarray(Wv), np.asarray(bv), np.asarray(Wo), np.asarray(bo),
    )
    xq_all = inputs_q.reshape(BLKS, BS, F)
    xkv_all = inputs_kv.reshape(BLKS, BS, F)
    wq_p = wq_p.astype(WNP)
    wk_p = wk_p.astype(WNP)
    wv_c = wv_c.astype(WNP)
    wo_c = wo_c.astype(WNP)

    if not _nc_cache:
        _nc_cache.append(build_program())
    nc = _nc_cache[0]

    in_maps = []
    for core in range(NCORES):
        cos_t, sin_t = _tables_for_core(core)
        in_maps.append(
            {
                "xq": np.ascontiguousarray(xq_all[core * BPC : (core + 1) * BPC]),
                "xkv": np.ascontiguousarray(xkv_all[core * BPC : (core + 1) * BPC]),
                "wq": wq_p, "wk": wk_p, "wv": wv_c, "wo": wo_c,
                "rt": rt.astype(WNP), "bq": bq_p, "bk": bk_p, "bo": bo_eff,
                "ident": np.eye(128, dtype=WNP),
                "ones": np.ones((1, 16, 64), WNP),
                "cos": cos_t.astype(WNP), "sin": sin_t.astype(WNP),
            }
        )
    res = run_bass_kernel_spmd(nc, in_maps, list(range(NCORES)))
    out = np.concatenate([res.results[i]["out"] for i in range(NCORES)], axis=0)
    return out.reshape(B, NB, BS, F)
